# revision 9
# baseline (speedup 1.0000x reference)
"""CorrelationLayer1D Trainium2 Bass kernel (v5).

Computes out[b, d, h, w] = sum_c x_1[b,c,h,w] * x2p[b,c,h,w+d] for d in [0, 41),
where x2p is x_2 width-padded by (8, 32).  Inputs [4,128,160,320] f32.

Sharding: data-parallel over H = 160 = 8*20 (correlation runs along W only, so
H-sharding needs no halo).  Per core, rows are processed in chunks of HC=10,
in row-groups of NG=3 (3,3,3,1).

Per (b, chunk):
  - fp32->bf16 casting loads (SWDGE/gpsimd), both tensors contiguous
    [C, hc*W] (6400B/partition descriptors).
  - Grams: per row, 3 matmuls with M=128/128/64 x1 stationaries (128-col
    stationaries trigger fast-weight-load) against clipped x2 windows; the
    (8, 32) zero-pad is realized by narrowing the two edge windows and
    memsetting the corresponding atlas j-columns.
  - STRIDED_GRAM=True (C'): each row's matmul writes PSUM with free stride
    NG (12B), interleaving the row-group j-major: group tile holds
    [M, j*ng + q].  Atlas copies and scratch stores are then PURE RECT, and
    the skewed band reload has (d, q)-contiguous runs of ng*D*2 = 246B with
    only 64/128 descriptors per (block, group).
    STRIDED_GRAM=False (A): classic row-major PSUM, rect copies per half,
    and per-(block, half) diag reloads with 82B runs (64*hc descriptors).
  - Back-end (emitted after the NEXT chunk's front so the PE never waits on
    the scratch round trip): per (group, block) ONE PE transpose of the
    contiguous [M, ng*D] sbig slice -> [ng*D, M] PSUM (partition = (q,d) or
    (d,q)), one cast copy into an [ng*D, W] f32 tile, and per group ONE
    rectangular store to out[b, :, h0+g0 : h0+g0+ng, :] via a 3-dim
    affine DRAM AP -- no [41, hc*W] assembly stage.
"""

import sys

import numpy as np

try:
    import concourse.bass as bass  # noqa: F401
except ImportError:
    sys.path.insert(0, "/opt/trn_rl_repo")

import concourse.bass as bass
import concourse.tile as tile
from concourse import bacc, masks, mybir
from concourse.ap import AP
from concourse.bass_utils import run_bass_kernel_spmd

MAX_DISP = 40
D = MAX_DISP + 1  # 41 displacements
PAD_L = 8
PAD_R = 32
B, C, H, W = 4, 128, 160, 320
N_CORES = 8
HS = H // N_CORES  # 20 h-rows per core
WBLOCKS = [(0, 128), (128, 128), (256, 64)]  # (w0, M); window width = M + 40
GW = 104  # band width per 64-partition half: 64 + MAX_DISP

F32 = mybir.dt.float32
BF16 = mybir.dt.bfloat16

STRIDED_GRAM = True


def _clip(w0, M):
    """Window [w0-8, w0+M+32) clipped to [0, W): returns (lo_col, jlo, jhi)."""
    lo = max(w0 - PAD_L, 0)
    hi = min(w0 + M + PAD_R, W)
    jlo = lo - (w0 - PAD_L)
    return lo, jlo, jlo + (hi - lo)


def build_kernel(b_dim=B, hs=HS, hc=10, ng=3, xin_bufs=3, strided=STRIDED_GRAM):
    assert hs % hc == 0
    nchunks = hs // hc
    groups = []
    g0 = 0
    while g0 < hc:
        groups.append((g0, min(ng, hc - g0)))
        g0 += ng

    nc = bacc.Bacc("TRN2", target_bir_lowering=False, debug=False)
    x1e = nc.declare_dram_parameter("x1", [b_dim, C, hs, W], F32, isOutput=False)
    x2e = nc.declare_dram_parameter("x2", [b_dim, C, hs, W], F32, isOutput=False)
    oute = nc.declare_dram_parameter("out", [b_dim, D, hs, W], F32, isOutput=True)

    # Per-group offsets into the (variable-ng) atlas / sbig layouts.
    goff_a = []  # atlas offset per group (C' layout: g-blocks of ng*GW)
    goff_s = []  # sbig offset per group (g-blocks of ng*D)
    o_a = o_s = 0
    for g0, g in groups:
        goff_a.append(o_a)
        goff_s.append(o_s)
        o_a += g * GW
        o_s += g * D
    assert o_a == hc * GW and o_s == hc * D

    with tile.TileContext(nc) as tc:
        with (
            tc.tile_pool(name="const", bufs=1) as const_pool,
            tc.tile_pool(name="xin", bufs=xin_bufs) as xin_pool,
            tc.tile_pool(name="atlas", bufs=2) as atlas_pool,
            tc.tile_pool(name="sbig", bufs=2) as sbig_pool,
            tc.tile_pool(name="asm", bufs=2) as asm_pool,
            tc.tile_pool(name="psum_g", bufs=2, space="PSUM") as psum_g,
            tc.tile_pool(name="psum_t", bufs=2, space="PSUM") as psum_t,
            tc.tile_pool(name="scratch", bufs=2, space="DRAM") as scratch_pool,
        ):
            identity = const_pool.tile([128, 128], BF16)
            masks.make_identity(nc, identity[:])

            def emit_front(b, ci):
                h0 = ci * hc
                x1b = xin_pool.tile(
                    [C, hc * W], BF16, tag="x1b", name=f"x1b_{b}_{ci}"
                )
                nc.gpsimd.dma_start(
                    x1b[:].rearrange("p (h w) -> p h w", w=W),
                    x1e[b, :, h0 : h0 + hc, :],
                )
                x2b = xin_pool.tile(
                    [C, hc * W], BF16, tag="x2b", name=f"x2b_{b}_{ci}"
                )
                nc.gpsimd.dma_start(
                    x2b[:].rearrange("p (h w) -> p h w", w=W),
                    x2e[b, :, h0 : h0 + hc, :],
                )

                atl = [
                    atlas_pool.tile(
                        [M, hc * GW], BF16, tag=f"A{kb}", name=f"A{kb}_{b}_{ci}"
                    )
                    for kb, (w0, M) in enumerate(WBLOCKS)
                ]

                # Zero the atlas j-columns the narrowed edge windows skip:
                # block 0 (j < 8) and block 2 (j >= 72), every h row.
                if strided:
                    for gi, (g0, g) in enumerate(groups):
                        nc.gpsimd.memset(
                            atl[0][0:64, goff_a[gi] : goff_a[gi] + g * PAD_L], 0.0
                        )
                        nc.gpsimd.memset(
                            atl[2][
                                0:64,
                                goff_a[gi] + g * 72 : goff_a[gi] + g * GW,
                            ],
                            0.0,
                        )
                else:
                    a0v = atl[0][0:64, :].rearrange("p (h j) -> p h j", j=GW)
                    nc.gpsimd.memset(a0v[:, :, 0:PAD_L], 0.0)
                    a2v = atl[2][0:64, :].rearrange("p (h j) -> p h j", j=GW)
                    nc.gpsimd.memset(a2v[:, :, 72:GW], 0.0)

                ncop = 0
                for gi, (g0, g) in enumerate(groups):
                    ps = [
                        psum_g.tile(
                            [M, ng * (M + MAX_DISP)],
                            F32,
                            tag=f"g{kb}",
                            name=f"g{kb}_{b}_{ci}_{g0}",
                        )
                        for kb, (w0, M) in enumerate(WBLOCKS)
                    ]
                    for q in range(g):
                        hh = g0 + q
                        for kb, (w0, M) in enumerate(WBLOCKS):
                            nw = M + MAX_DISP
                            lo, jlo, jhi = _clip(w0, M)
                            rhs = x2b[:, hh * W + lo : hh * W + lo + (jhi - jlo)]
                            if strided:
                                pb = ps[kb][:]
                                out_ap = AP(
                                    tensor=pb.tensor,
                                    offset=pb.offset + g * jlo + q,
                                    ap=[list(pb.ap[0]), [g, jhi - jlo]],
                                )
                            else:
                                out_ap = ps[kb][:, q * nw + jlo : q * nw + jhi]
                            nc.tensor.matmul(
                                out_ap,
                                x1b[:, hh * W + w0 : hh * W + w0 + M],
                                rhs,
                                start=True,
                                stop=True,
                            )
                    for kb, (w0, M) in enumerate(WBLOCKS):
                        nw = M + MAX_DISP
                        lo, jlo, jhi = _clip(w0, M)
                        for hf in range(M // 64):
                            # This half's j-window is [64*hf, 64*hf+GW); clip
                            # to the written region [jlo, jhi).
                            cl = max(jlo, 64 * hf)
                            ch = min(jhi, 64 * hf + GW)
                            if strided:
                                src = ps[kb][
                                    64 * hf : 64 * hf + 64, g * cl : g * ch
                                ]
                                dst = atl[kb][
                                    64 * hf : 64 * hf + 64,
                                    goff_a[gi]
                                    + g * (cl - 64 * hf) : goff_a[gi]
                                    + g * (ch - 64 * hf),
                                ]
                            else:
                                psv = ps[kb][:].rearrange(
                                    "p (q j) -> p q j", j=nw
                                )
                                src = psv[64 * hf : 64 * hf + 64, :, cl:ch]
                                av = atl[kb][64 * hf : 64 * hf + 64, :].rearrange(
                                    "p (h j) -> p h j", j=GW
                                )
                                dst = av[
                                    :, g0 : g0 + g, cl - 64 * hf : ch - 64 * hf
                                ]
                            if ncop % 2 == 0:
                                nc.vector.tensor_copy(dst, src)
                            else:
                                nc.scalar.copy(dst, src)
                            ncop += 1

                sbig = []
                scrs = []
                for kb, (w0, M) in enumerate(WBLOCKS):
                    scr = scratch_pool.tile(
                        [M, hc * GW], BF16, tag=f"scr{kb}", name=f"scr{kb}_{b}_{ci}"
                    )
                    nc.sync.dma_start(scr[:], atl[kb][:])
                    scrs.append(scr)
                    sbig.append(
                        sbig_pool.tile(
                            [M, hc * D], BF16, tag=f"sb{kb}", name=f"sb{kb}_{b}_{ci}"
                        )
                    )

                nrl = 0
                if strided:
                    for kb, (w0, M) in enumerate(WBLOCKS):
                        sap = scrs[kb][:]
                        for gi, (g0, g) in enumerate(groups):
                            dims = [[hc * GW + g, 64], [1, g * D]]
                            if M == 128:
                                dims = [[64 * hc * GW, 2]] + dims
                            diag = AP(
                                tensor=sap.tensor,
                                offset=sap.offset + goff_a[gi],
                                ap=dims,
                            )
                            eng = nc.scalar if nrl % 2 == 0 else nc.sync
                            eng.dma_start(
                                sbig[kb][
                                    0:M, goff_s[gi] : goff_s[gi] + g * D
                                ],
                                diag,
                            )
                            nrl += 1
                else:
                    for kb, (w0, M) in enumerate(WBLOCKS):
                        sap = scrs[kb][:]
                        for hf in range(M // 64):
                            diag = AP(
                                tensor=sap.tensor,
                                offset=sap.offset + 64 * hf * hc * GW,
                                ap=[[hc * GW + 1, 64], [GW, hc], [1, D]],
                            )
                            dstp = sbig[kb][
                                64 * hf : 64 * hf + 64, :
                            ].rearrange("p (h d) -> p h d", d=D)
                            eng = nc.scalar if nrl % 2 == 0 else nc.sync
                            eng.dma_start(dstp, diag)
                            nrl += 1
                return sbig

            def emit_back(b, ci, sbig):
                h0 = ci * hc
                ncop = 0
                for gi, (g0, g) in enumerate(groups):
                    asm = asm_pool.tile(
                        [g * D, W], F32, tag=f"as{g0}", name=f"as{g0}_{b}_{ci}"
                    )
                    for kb, (w0, M) in enumerate(WBLOCKS):
                        t_ps = psum_t.tile(
                            [ng * D, M],
                            BF16,
                            tag="t_ps",
                            name=f"t_ps_{b}_{ci}_{g0}_{kb}",
                        )
                        nc.tensor.matmul(
                            t_ps[0 : g * D, 0:M],
                            sbig[kb][0:M, goff_s[gi] : goff_s[gi] + g * D],
                            identity[0:M, 0:M],
                            start=True,
                            stop=True,
                            is_transpose=True,
                        )
                        dst = asm[:, w0 : w0 + M]
                        if ncop % 2 == 0:
                            nc.scalar.copy(dst, t_ps[0 : g * D, 0:M])
                        else:
                            nc.vector.tensor_copy(dst, t_ps[0 : g * D, 0:M])
                        ncop += 1
                    # Store [g*D, W] straight to out[b, :, h0+g0 : +g, :].
                    # Partition order is (d, q) for strided (C') sbig layout,
                    # (q, d) for row-major (A).
                    ob = oute[b, 0:D, h0 + g0 : h0 + g0 + g, :]
                    oap = ob  # [[hs*W, D], [W, g], [1, W]]
                    if not strided:
                        oap = AP(
                            tensor=ob.tensor,
                            offset=ob.offset,
                            ap=[[W, g], [hs * W, D], [1, W]],
                        )
                    store_eng = nc.gpsimd if gi % 2 == 0 else nc.sync
                    store_eng.dma_start(oap, asm[:])

            prev = None
            for b in range(b_dim):
                for ci in range(nchunks):
                    sbig = emit_front(b, ci)
                    if prev is not None:
                        emit_back(*prev)
                    prev = (b, ci, sbig)
            emit_back(*prev)

    nc.finalize()
    return nc


_compiled = {}


def _get_kernel(b_dim, hs):
    key = (b_dim, hs)
    if key not in _compiled:
        _compiled[key] = build_kernel(b_dim, hs)
    return _compiled[key]


def kernel(x_1: np.ndarray, x_2: np.ndarray) -> np.ndarray:
    assert x_1.shape == (B, C, H, W) and x_2.shape == (B, C, H, W)
    x_1 = np.ascontiguousarray(x_1, dtype=np.float32)
    x_2 = np.ascontiguousarray(x_2, dtype=np.float32)
    nc = _get_kernel(B, HS)
    in_maps = [
        {
            "x1": np.ascontiguousarray(x_1[:, :, i * HS : (i + 1) * HS, :]),
            "x2": np.ascontiguousarray(x_2[:, :, i * HS : (i + 1) * HS, :]),
        }
        for i in range(N_CORES)
    ]
    res = run_bass_kernel_spmd(nc, in_maps, core_ids=list(range(N_CORES))).results
    out = np.concatenate([res[i]["out"] for i in range(N_CORES)], axis=2)
    return out


# revision 10
# speedup vs baseline: 1.1884x; 1.1884x over previous
"""CorrelationLayer1D Trainium2 Bass kernel (v6).

Computes out[b, d, h, w] = sum_c x_1[b,c,h,w] * x2p[b,c,h,w+d] for d in [0, 41),
where x2p is x_2 width-padded by (8, 32).  Inputs [4,128,160,320] f32.

Sharding: data-parallel over H = 160 = 8*20 (correlation runs along W only, so
H-sharding needs no halo).  Per core: chunks of HC=10 rows, row-pairs NG=2.

Key structure (per (b, chunk)):
  - fp32->bf16 casting loads (SWDGE/gpsimd), contiguous [C, hc*W].
  - Grams: per row, 3 matmuls with M=128/128/64 x1 stationaries against
    clipped x2 windows (edge pads realized by narrowing + atlas memsets).
    Each row's matmul writes PSUM with free stride NG=2 (8B), interleaving
    the row-pair j-major: the group tile holds [M, j*2 + q].  Strided PSUM
    drain was HW-measured to cost ~0 extra.
  - The atlas keeps the FULL j in [0, M+40) window per block (no 64-half
    split), so the band shear j = p + d is uniform over all 128 partitions:
    scr addr = p*(rowpitch + 2) + group*2*nw + (d*2 + q).  Atlas copies are
    pure rectangles; blocks 0+1 share one [128, 2*hc*168] tile.
  - DMA instructions per chunk: 2 casting loads, 2 rect scratch stores
    (blk01, blk2), 3 skewed reloads (one per block, groups merged into a
    3-dim AP, (d,q)-contiguous 164B runs, 128 descriptors), 5 output
    stores.  Issuing-queue occupancy was the v5 bottleneck at 21 instrs.
  - Back-end (emitted after the NEXT chunk's front so the PE never waits on
    the scratch round trip): per (row-pair, block) one PE transpose of the
    contiguous [M, 2*D] sbig slice -> [2*41, M] PSUM (partition = (d, q)),
    one cast copy into the row-pair's [82, W] f32 tile, then one store
    straight to out[b, :, h0+2g : h0+2g+2, :] via a 3-dim affine DRAM AP.
"""

import sys

import numpy as np

try:
    import concourse.bass as bass  # noqa: F401
except ImportError:
    sys.path.insert(0, "/opt/trn_rl_repo")

import concourse.bass as bass
import concourse.tile as tile
from concourse import bacc, masks, mybir
from concourse.ap import AP
from concourse.bass_utils import run_bass_kernel_spmd

MAX_DISP = 40
D = MAX_DISP + 1  # 41 displacements
PAD_L = 8
PAD_R = 32
B, C, H, W = 4, 128, 160, 320
N_CORES = 8
HS = H // N_CORES  # 20 h-rows per core
WBLOCKS = [(0, 128), (128, 128), (256, 64)]  # (w0, M); window width nw = M + 40

F32 = mybir.dt.float32
BF16 = mybir.dt.bfloat16


def _clip(w0, M):
    """Window [w0-8, w0+M+32) clipped to [0, W): (x2_lo_col, jlo, jhi)."""
    lo = max(w0 - PAD_L, 0)
    hi = min(w0 + M + PAD_R, W)
    jlo = lo - (w0 - PAD_L)
    return lo, jlo, jlo + (hi - lo)


def build_kernel(b_dim=B, hs=HS, hc=10, xin_bufs=3):
    ng = 2
    assert hs % hc == 0 and hc % ng == 0
    nchunks = hs // hc
    ngrp = hc // ng
    NW0 = 168  # blocks 0/1 window width
    NW2 = 104  # block 2 window width
    P01 = 2 * hc * NW0  # atl01/scr01 row pitch in elements (two kb blocks)

    nc = bacc.Bacc("TRN2", target_bir_lowering=False, debug=False)
    x1e = nc.declare_dram_parameter("x1", [b_dim, C, hs, W], F32, isOutput=False)
    x2e = nc.declare_dram_parameter("x2", [b_dim, C, hs, W], F32, isOutput=False)
    oute = nc.declare_dram_parameter("out", [b_dim, D, hs, W], F32, isOutput=True)

    with tile.TileContext(nc) as tc:
        with (
            tc.tile_pool(name="const", bufs=1) as const_pool,
            tc.tile_pool(name="xin", bufs=xin_bufs) as xin_pool,
            tc.tile_pool(name="atlas", bufs=2) as atlas_pool,
            tc.tile_pool(name="sbig", bufs=2) as sbig_pool,
            tc.tile_pool(name="asm", bufs=2) as asm_pool,
            tc.tile_pool(name="psum_g", bufs=2, space="PSUM") as psum_g,
            tc.tile_pool(name="psum_t", bufs=2, space="PSUM") as psum_t,
            tc.tile_pool(name="scratch", bufs=2, space="DRAM") as scratch_pool,
        ):
            identity = const_pool.tile([128, 128], BF16)
            masks.make_identity(nc, identity[:])

            def emit_front(b, ci):
                h0 = ci * hc
                x1b = xin_pool.tile(
                    [C, hc * W], BF16, tag="x1b", name=f"x1b_{b}_{ci}"
                )
                nc.gpsimd.dma_start(
                    x1b[:].rearrange("p (h w) -> p h w", w=W),
                    x1e[b, :, h0 : h0 + hc, :],
                )
                x2b = xin_pool.tile(
                    [C, hc * W], BF16, tag="x2b", name=f"x2b_{b}_{ci}"
                )
                nc.gpsimd.dma_start(
                    x2b[:].rearrange("p (h w) -> p h w", w=W),
                    x2e[b, :, h0 : h0 + hc, :],
                )

                # Atlas: blocks 0+1 in one [128, P01] tile (kb-region pitch
                # hc*NW0), block 2 in [64, hc*NW2]; both group-blocked with
                # the row-pair interleaved: col = kb_off + g*(ng*nw) + j*ng + q.
                atl01 = atlas_pool.tile(
                    [128, P01], BF16, tag="A01", name=f"A01_{b}_{ci}"
                )
                atl2 = atlas_pool.tile(
                    [64, hc * NW2], BF16, tag="A2", name=f"A2_{b}_{ci}"
                )
                # Zero the j-columns the narrowed edge windows skip:
                # block 0: j in [0, 8); block 2: j in [72, 104).  One strided
                # memset each, spanning all groups.
                a01v = atl01[:].rearrange("p (g j) -> p g j", j=ng * NW0)
                nc.gpsimd.memset(a01v[:, 0:ngrp, 0 : ng * PAD_L], 0.0)
                a2v = atl2[:].rearrange("p (g j) -> p g j", j=ng * NW2)
                nc.gpsimd.memset(a2v[:, :, ng * 72 : ng * NW2], 0.0)

                ncop = 0
                for gi in range(ngrp):
                    g0 = gi * ng
                    ps = [
                        psum_g.tile(
                            [M, ng * (M + MAX_DISP)],
                            F32,
                            tag=f"g{kb}",
                            name=f"g{kb}_{b}_{ci}_{gi}",
                        )
                        for kb, (w0, M) in enumerate(WBLOCKS)
                    ]
                    for q in range(ng):
                        hh = g0 + q
                        for kb, (w0, M) in enumerate(WBLOCKS):
                            lo, jlo, jhi = _clip(w0, M)
                            pb = ps[kb][:]
                            out_ap = AP(
                                tensor=pb.tensor,
                                offset=pb.offset + ng * jlo + q,
                                ap=[list(pb.ap[0]), [ng, jhi - jlo]],
                            )
                            nc.tensor.matmul(
                                out_ap,
                                x1b[:, hh * W + w0 : hh * W + w0 + M],
                                x2b[:, hh * W + lo : hh * W + lo + (jhi - jlo)],
                                start=True,
                                stop=True,
                            )
                    for kb, (w0, M) in enumerate(WBLOCKS):
                        nw = M + MAX_DISP
                        lo, jlo, jhi = _clip(w0, M)
                        src = ps[kb][0:M, ng * jlo : ng * jhi]
                        if kb < 2:
                            base = kb * hc * NW0 + gi * ng * NW0
                            dst = atl01[:, base + ng * jlo : base + ng * jhi]
                        else:
                            base = gi * ng * NW2
                            dst = atl2[:, base + ng * jlo : base + ng * jhi]
                        if ncop % 2 == 0:
                            nc.vector.tensor_copy(dst, src)
                        else:
                            nc.scalar.copy(dst, src)
                        ncop += 1

                scr01 = scratch_pool.tile(
                    [128, P01], BF16, tag="scr01", name=f"scr01_{b}_{ci}"
                )
                nc.sync.dma_start(scr01[:], atl01[:])
                scr2 = scratch_pool.tile(
                    [64, hc * NW2], BF16, tag="scr2", name=f"scr2_{b}_{ci}"
                )
                nc.sync.dma_start(scr2[:], atl2[:])

                # Skewed band reloads: band j = p + d, so
                # addr = p*(pitch + ng) + kb_off + g*(ng*nw) + (d*ng + q).
                sbig = []
                for kb, (w0, M) in enumerate(WBLOCKS):
                    sb = sbig_pool.tile(
                        [M, hc * D], BF16, tag=f"sb{kb}", name=f"sb{kb}_{b}_{ci}"
                    )
                    if kb < 2:
                        sap = scr01[:]
                        dims = [
                            [P01 + ng, 128],
                            [ng * NW0, ngrp],
                            [1, ng * D],
                        ]
                        off = sap.offset + kb * hc * NW0
                    else:
                        sap = scr2[:]
                        dims = [
                            [hc * NW2 + ng, 64],
                            [ng * NW2, ngrp],
                            [1, ng * D],
                        ]
                        off = sap.offset
                    diag = AP(tensor=sap.tensor, offset=off, ap=dims)
                    eng = nc.scalar if kb % 2 == 0 else nc.sync
                    eng.dma_start(sb[:], diag)
                    sbig.append(sb)
                return sbig

            def emit_back(b, ci, sbig):
                h0 = ci * hc
                ncop = 0
                for gi in range(hc // ng):
                    asm = asm_pool.tile(
                        [ng * D, W], F32, tag=f"as{gi}", name=f"as{gi}_{b}_{ci}"
                    )
                    for kb, (w0, M) in enumerate(WBLOCKS):
                        t_ps = psum_t.tile(
                            [ng * D, 128],
                            BF16,
                            tag="t_ps",
                            name=f"t_ps_{b}_{ci}_{gi}_{kb}",
                        )
                        nc.tensor.matmul(
                            t_ps[:, 0:M],
                            sbig[kb][0:M, gi * ng * D : (gi + 1) * ng * D],
                            identity[0:M, 0:M],
                            start=True,
                            stop=True,
                            is_transpose=True,
                        )
                        dst = asm[:, w0 : w0 + M]
                        if ncop % 2 == 0:
                            nc.scalar.copy(dst, t_ps[:, 0:M])
                        else:
                            nc.vector.tensor_copy(dst, t_ps[:, 0:M])
                        ncop += 1
                    # Partition p = d*ng + q -> out[b, d, h0+gi*ng+q, :].
                    ob = oute[b, 0:D, h0 + gi * ng : h0 + gi * ng + ng, :]
                    store_eng = nc.sync if gi % 2 == 0 else nc.gpsimd
                    store_eng.dma_start(ob, asm[:])

            prev = None
            for b in range(b_dim):
                for ci in range(nchunks):
                    sbig = emit_front(b, ci)
                    if prev is not None:
                        emit_back(*prev)
                    prev = (b, ci, sbig)
            emit_back(*prev)

    nc.finalize()
    return nc


_compiled = {}


def _get_kernel(b_dim, hs):
    key = (b_dim, hs)
    if key not in _compiled:
        _compiled[key] = build_kernel(b_dim, hs)
    return _compiled[key]


def kernel(x_1: np.ndarray, x_2: np.ndarray) -> np.ndarray:
    assert x_1.shape == (B, C, H, W) and x_2.shape == (B, C, H, W)
    x_1 = np.ascontiguousarray(x_1, dtype=np.float32)
    x_2 = np.ascontiguousarray(x_2, dtype=np.float32)
    nc = _get_kernel(B, HS)
    in_maps = [
        {
            "x1": np.ascontiguousarray(x_1[:, :, i * HS : (i + 1) * HS, :]),
            "x2": np.ascontiguousarray(x_2[:, :, i * HS : (i + 1) * HS, :]),
        }
        for i in range(N_CORES)
    ]
    res = run_bass_kernel_spmd(nc, in_maps, core_ids=list(range(N_CORES))).results
    out = np.concatenate([res[i]["out"] for i in range(N_CORES)], axis=2)
    return out
